# revision 10
# baseline (speedup 1.0000x reference)
"""Distributed Trainium2 kernel for nn_ArcTransformer (8 NeuronCores).

Algorithmic structure exploited (fixed problem shapes, V=16 vocab):
  * Every per-token q/k/v vector depends only on the token id (the MoE
    "compose" is position-independent), so the dense per-token expert MLP
    collapses to the 16 vocab rows.
  * Causal softmax attention over positions collapses to a cumulative
    token-count weighted sum over the 16 vocab classes:
        attn[t] = sum_v E[tok_t,v] * C[t,v] * v16[v] / sum_v E[tok_t,v]*C[t,v]
    with E = exp(scores between vocab rows), C = causal inclusive count
    of each vocab class up to position t.
  * Output projection + LM head fold into a single [16,16] matrix per head.

Sharding: data-parallel over batch, one full 2048-token row per core
(M=2). The kernel is tiny, so steady-state execute time is dominated by
per-core NEFF launch overhead, which the runtime serializes — measured
sustained per-execution time scales with core count (8 cores ~2.0 ms,
4 ~1.9 ms, 2 ~1.3 ms), so the widest shard that keeps batch-parallel
SPMD wins. Each core walks its row in four 512-position tiles, chaining
the class-count scan carry across tiles on device (carry-in at a row
start is zero). The only cross-head reduction is local (one K=128
matmul); no inter-core collective is needed — an on-device AllGather of
the outputs was measured SLOWER (per-iteration cross-core rendezvous).
Each core returns the logits for its row; the host concatenates.

Executable I/O is exactly ONE input operand and ONE output operand per
core (the unused partition-id input is disabled) — per-launch cost also
scales with the number of DRAM bindings, so everything the device needs,
tables AND constant masks, rides in one packed [128, 4546] fp16 input
(payload size is irrelevant next to binding count; the masks also drop
the gpsimd iota/affine_select ops, leaving 3 engine queues: sync DMA,
vector, tensor). Packed columns:
  *    0:16   rows 0:128 — folded value->logit table VO [128, 16];
  *   16:17   rows 0:128 — carry-in class counts (zero at row starts);
  *   17:145  rows 0:16  — per-head exp-score rows E (estk [16, 128]);
  *  145:161  rows 0:16  — folded residual logits embed @ head_w.T;
  *  161:2209 row  0     — the row's 2048 raw token ids as fp16;
  * 2210:4258 rows 0:16  — partition index (one-hot compare operand);
  * 4260:4388 rows 0:16  — tilew: eye(16) tiled across heads;
  * 4388:4516 rows 0:8   — bcw[h, p] = (p // 16 == h);
  * 4516:4524 rows 0:128 — denw = bcw.T;
  * 4524:4540 row  0     — ones[1, 16] (K=1 id-broadcast weights).
One-hot tokens: PE K=1 broadcast of the ids row, is_equal against the
partition-index block. Compute is all-fp32; DRAM I/O is fp16.

Device layout: [128, 512] tiles; partition p = h*16+v for head h and
vocab v; free dim = position within the current 512-wide tile.
"""

import sys

import numpy as np

sys.path.insert(0, "/opt/trn_rl_repo")

import jax  # noqa: E402

# The bass_exec HLO is deterministic, but each run_bass_kernel_spmd call
# jits a fresh closure, so the in-memory executable cache always misses
# and every call re-runs the ~350 ms neuronx compile hook. The persistent
# cache dedupes on HLO bytes and turns repeat calls into a disk hit.
for _opt, _val in (
    ("jax_compilation_cache_dir", "/tmp/jax_comp_cache"),
    ("jax_persistent_cache_min_compile_time_secs", 0.0),
    ("jax_persistent_cache_min_entry_size_bytes", 0),
):
    try:
        jax.config.update(_opt, _val)
    except Exception:
        pass

from concourse import bacc, mybir, tile  # noqa: E402
from concourse.bass_utils import run_bass_kernel_spmd  # noqa: E402

B, T, V, D = 2, 2048, 16, 512
NH, DH, P = 8, 64, 16
BT = B * T           # 4096 tokens
NCORES = 2           # data-parallel over batch: one row per core
CW = BT // NCORES    # 2048 tokens per core
TILE = 512           # free-dim tile width
NTILE = CW // TILE
F32 = mybir.dt.float32
F16 = mybir.dt.float16

# packed [128, INW] input column offsets (see module docstring)
IN_VO, IN_BASE, IN_ESTK, IN_XLT, IN_IDS = 0, 16, 17, 145, 161
IN_VIS = IN_IDS + CW + 1        # 2210
IN_TILEW = IN_VIS + CW + 2      # 4260
IN_BCW = IN_TILEW + 128         # 4388
IN_DENW = IN_BCW + 128          # 4516
IN_ONES = IN_DENW + NH          # 4524
INW = IN_ONES + V + 6           # 4546

_STATE = {}


def _build_nc():
    # target_bir_lowering=True lowers through AwsNeuronCustomNativeKernel:
    # stock neuronx-cc inlines the BIR into a regular NEFF instead of the
    # bass_exec custom-call NEFF wrapper. Measured: the wrapper pays a
    # ~0.5 ms/exec per-launch runtime tax this path does not (0.48 ms vs
    # 0.92 ms sustained per execution at 2 cores, identical numerics).
    nc = bacc.Bacc("TRN2", target_bir_lowering=True, debug=False,
                   num_devices=NCORES, enable_partition_id=False)

    inp_d = nc.declare_dram_parameter("inp", [128, INW], F16, isOutput=False)
    out_ext = nc.declare_dram_parameter("out", [V, CW], F16, isOutput=True)

    eq = mybir.AluOpType.is_equal
    add = mybir.AluOpType.add

    with tile.TileContext(nc) as tc:
        with (
            tc.tile_pool(name="sb", bufs=1) as sb,
            tc.tile_pool(name="ps", bufs=1, space="PSUM") as ps,
        ):
            # fp16 end to end on the PE datapath: one-hots and counts are
            # exact in fp16 (integers <= 2048), tables are fp16 payload
            # already, and PE fp16 matmuls avoid the 4-pass fp32r mode.
            inp_h = sb.tile([128, INW], F16)
            nc.sync.dma_start(inp_h[:], inp_d[:])

            vo = inp_h[0:128, IN_VO:IN_VO + V]        # value->logit [128,16]
            base128 = inp_h[0:128, IN_BASE:IN_BASE + 1]  # carry-in counts
            estk = inp_h[0:16, IN_ESTK:IN_ESTK + 128]    # E_h[u,v] at h*16+v
            xlt = inp_h[0:16, IN_XLT:IN_XLT + V]         # embed @ head_w.T
            ids_h = inp_h[0:1, IN_IDS:IN_IDS + CW]       # token ids, fp16
            vis = inp_h[0:16, IN_VIS:IN_VIS + CW]        # partition index
            tilew = inp_h[0:16, IN_TILEW:IN_TILEW + 128]
            bcw = inp_h[0:NH, IN_BCW:IN_BCW + 128]
            denw = inp_h[0:128, IN_DENW:IN_DENW + NH]
            ones16 = inp_h[0:1, IN_ONES:IN_ONES + V]

            cnt_all = sb.tile([128, CW], F16)
            g_all = sb.tile([128, CW], F16)
            gn_all = sb.tile([128, CW], F16)
            out_all = sb.tile([V, CW], F16)
            zero128 = sb.tile([128, TILE], F16)
            nc.vector.memset(zero128[:], 0.0)

            # walk the row in 512-wide tiles, chaining the count-scan carry
            carry = base128
            for j in range(NTILE):
                s = slice(j * TILE, (j + 1) * TILE)

                # one-hot tokens: broadcast ids across 16 partitions (PE,
                # K=1), compare against the partition index
                idsb_ps = ps.tile([V, TILE], F32)
                nc.tensor.matmul(idsb_ps[:], ones16, ids_h[0:1, s])
                oh = sb.tile([V, TILE], F16)
                nc.vector.tensor_tensor(out=oh[:], in0=idsb_ps[:],
                                        in1=vis[0:16, s], op=eq)

                # broadcast the one-hot to all 8 head blocks (PE), then run
                # the inclusive count scan at 128 partitions straight out
                # of PSUM; fp16 counts are exact (integers <= 2048)
                ohb_ps = ps.tile([128, TILE], F32)
                nc.tensor.matmul(ohb_ps[:], tilew, oh[:])
                cnt = cnt_all[0:128, s]
                with nc.allow_low_precision(
                        reason="counts <= 2048 are exact in fp16"):
                    nc.vector.tensor_tensor_scan(
                        out=cnt, data0=ohb_ps[:], data1=zero128[:],
                        initial=carry, op0=add, op1=add)
                carry = cnt_all[0:128, (j + 1) * TILE - 1:(j + 1) * TILE]

                # G[h*16+v, t] = E_h[tok_t, v] * C[t, v]
                erow_ps = ps.tile([128, TILE], F32)
                nc.tensor.matmul(erow_ps[:], estk, oh[:])
                log_ps = ps.tile([V, TILE], F32)
                nc.tensor.matmul(log_ps[:], xlt, oh[:], start=True,
                                 stop=False)
                g = g_all[0:128, s]
                nc.vector.tensor_mul(g, erow_ps[:], cnt)

                # softmax denominator per head, reciprocal, broadcast back
                # (den <= t+1 <= 2048, so 1/den stays in fp16's normal
                # range and the fp16 reciprocal is safe)
                den_ps = ps.tile([NH, TILE], F32)
                nc.tensor.matmul(den_ps[:], denw, g)
                rec = sb.tile([NH, TILE], F16)
                with nc.allow_low_precision(
                        reason="1/den in [4.9e-4, 1], fp16 rel err 5e-4"):
                    nc.vector.reciprocal(rec[:], den_ps[:])
                bc_ps = ps.tile([128, TILE], F32)
                nc.tensor.matmul(bc_ps[:], bcw, rec[:])
                gn = gn_all[0:128, s]
                nc.vector.tensor_mul(gn, g, bc_ps[:])

                # logits[e,t] = sum_{h,v} VO[hv,e] * Gn[hv,t] + XL[tok_t,e]
                # (the XL term was accumulated into log_ps up front);
                # downcast on the vector engine — DMA cannot read PSUM, and
                # vector keeps the NEFF at 3 engine queues
                nc.tensor.matmul(log_ps[:], vo, gn, start=False, stop=True)
                nc.vector.tensor_tensor(out=out_all[0:V, s], in0=log_ps[:],
                                        in1=zero128[0:V, :], op=add)

            nc.sync.dma_start(out_ext[:], out_all[:])

    nc.compile()
    return nc


def _prep_inputs(inputs):
    ids = np.asarray(inputs["input_ids"]).astype(np.int64).reshape(BT)
    embed = np.asarray(inputs["embed"], dtype=np.float32)
    ln_g = np.asarray(inputs["ln_g"], dtype=np.float32)
    ln_b = np.asarray(inputs["ln_b"], dtype=np.float32)
    w1 = np.asarray(inputs["w1"], dtype=np.float32)
    w2 = np.asarray(inputs["w2"], dtype=np.float32)
    o_w = np.asarray(inputs["o_w"], dtype=np.float32)
    head_w = np.asarray(inputs["head_w"], dtype=np.float32)

    # LayerNorm of the 16 vocab embedding rows
    mu = embed.mean(axis=-1, keepdims=True)
    var = ((embed - mu) ** 2).mean(axis=-1, keepdims=True)
    h16 = (embed - mu) / np.sqrt(var + 1e-5) * ln_g + ln_b
    A = h16.reshape(V * NH, DH)                 # [128, 64] per-head rows

    scale = 1.0 / np.sqrt(DH)

    # expert MLP of the 16 vocab rows — shared by q/k/v (gate-independent)
    hmid = A @ w1.reshape(P * DH, DH).T         # [128, P*64]
    s = hmid * (1.0 / (1.0 + np.exp(-hmid)))    # silu
    s_p = np.ascontiguousarray(
        s.reshape(V * NH, P, DH).transpose(1, 0, 2))   # [P, 128, 64]
    outm = s_p @ w2.transpose(0, 2, 1)          # [P, 128, 64]

    def compose16(proto, gate):
        logits = (A @ np.asarray(proto, np.float32).T) * scale \
            - np.asarray(gate, np.float32)      # [128, P]
        w = np.where(logits > 1e-6, logits, 0.0).astype(np.float32)
        out = np.einsum("pxe,xp->xe", outm, w)  # [128, 64]
        return out.reshape(V, NH, DH).astype(np.float32)

    q16 = compose16(inputs["proto_q"], inputs["gate_q"])
    k16 = compose16(inputs["proto_k"], inputs["gate_k"])
    v16 = compose16(inputs["proto_v"], inputs["gate_v"])

    # per-head exp-score tables and folded value->logits matrices
    E_list, VO_list = [], []
    for h in range(NH):
        S = (q16[:, h, :] @ k16[:, h, :].T) * scale        # [16, 16]
        E_list.append(
            np.exp(S - S.max(axis=1, keepdims=True)).astype(np.float32))
        OW = o_w.T[h * DH:(h + 1) * DH, :] @ head_w.T       # [64, 16]
        VO_list.append((v16[:, h, :] @ OW).astype(np.float32))

    XL = embed @ head_w.T                       # [16, 16] residual-path logits

    estk = np.concatenate(E_list, axis=1)       # [16, 128]: E_h[u,v] @ h*16+v
    vo_st = np.concatenate(VO_list, axis=0)     # [128, 16]
    ids16 = ids.astype(np.float16).reshape(NCORES, CW)

    # constant masks, shipped once inside the packed input
    vis_col = np.arange(V, dtype=np.float16)[:, None]
    tilew = np.tile(np.eye(V, dtype=np.float16), (1, NH))    # [16, 128]
    bcw = np.zeros((NH, 128), np.float16)
    for h in range(NH):
        bcw[h, h * V:(h + 1) * V] = 1.0
    denw = bcw.T.copy()                                      # [128, 8]

    in_maps = []
    for i in range(NCORES):
        # each chunk starts at a batch-row boundary -> zero carry-in
        # (kept as an input column so the kernel stays general)
        inp = np.zeros((128, INW), np.float16)
        inp[:, IN_VO:IN_VO + V] = vo_st.astype(np.float16)
        inp[:, IN_BASE] = 0.0
        inp[0:V, IN_ESTK:IN_ESTK + 128] = estk.astype(np.float16)
        inp[0:V, IN_XLT:IN_XLT + V] = XL.astype(np.float16)
        inp[0, IN_IDS:IN_IDS + CW] = ids16[i]
        inp[0:V, IN_VIS:IN_VIS + CW] = np.broadcast_to(vis_col, (V, CW))
        inp[0:V, IN_TILEW:IN_TILEW + 128] = tilew
        inp[0:NH, IN_BCW:IN_BCW + 128] = bcw
        inp[:, IN_DENW:IN_DENW + NH] = denw
        inp[0, IN_ONES:IN_ONES + V] = 1.0
        in_maps.append({"inp": inp})
    return in_maps


def _unshard(per_core_out):
    # core i holds logits (vocab-major) for tokens [i*CW, (i+1)*CW)
    full = np.concatenate(
        [np.asarray(o, dtype=np.float32) for o in per_core_out], axis=1)
    return np.ascontiguousarray(full.T.reshape(B, T, V)).astype(np.float32)


def _make_runner(nc):
    """Persistent fast-dispatch executable for repeat kernel() calls —
    same bass_exec primitive run_bass_kernel_spmd lowers to, but compiled
    once and reused, so a second call skips the fresh-jit path."""
    import jax
    from jax.sharding import Mesh, PartitionSpec
    from jax.experimental.shard_map import shard_map
    from concourse import mybir as _mybir
    from concourse.bass2jax import (
        _bass_exec_p, fast_dispatch_compile, install_neuronx_cc_hook)

    install_neuronx_cc_hook()
    in_names, out_names, out_avals, zero_outs = [], [], [], []
    for alloc in nc.m.functions[0].allocations:
        if not isinstance(alloc, _mybir.MemoryLocationSet):
            continue
        name = alloc.memorylocations[0].name
        if alloc.kind == "ExternalInput":
            in_names.append(name)
        elif alloc.kind == "ExternalOutput":
            shape = tuple(alloc.tensor_shape)
            dtype = _mybir.dt.np(alloc.dtype)
            out_names.append(name)
            out_avals.append(jax.core.ShapedArray(shape, dtype))
            zero_outs.append(np.zeros(shape, dtype))
    n_params = len(in_names)
    all_names = in_names + out_names
    donate = tuple(range(n_params, n_params + len(out_avals)))

    def _body(*args):
        return tuple(_bass_exec_p.bind(
            *args, out_avals=tuple(out_avals), in_names=tuple(all_names),
            out_names=tuple(out_names), lowering_input_output_aliases=(),
            sim_require_finite=True, sim_require_nnan=True, nc=nc))

    from jax.sharding import NamedSharding
    mesh = Mesh(np.asarray(jax.devices()[:NCORES]), ("core",))
    shard = NamedSharding(mesh, PartitionSpec("core"))
    n_args = n_params + len(out_avals)
    jitted = jax.jit(
        shard_map(_body, mesh=mesh,
                  in_specs=(PartitionSpec("core"),) * n_args,
                  out_specs=(PartitionSpec("core"),) * len(out_names),
                  check_rep=False),
        donate_argnums=donate, keep_unused=True)

    def _place(arrays):
        return [jax.device_put(a, shard) for a in arrays]

    def run(in_maps):
        concat_in = [np.concatenate([np.asarray(m[name]) for m in in_maps], 0)
                     for name in in_names]
        zeros = [np.zeros((NCORES * z.shape[0], *z.shape[1:]), z.dtype)
                 for z in zero_outs]
        outs = compiled(*_place(concat_in), *_place(zeros))
        out = np.asarray(outs[0])
        return [out[i * V:(i + 1) * V] for i in range(NCORES)]

    example_in = _place([np.zeros((NCORES * 128, INW), np.float16)])
    example_zeros = _place(
        [np.zeros((NCORES * z.shape[0], *z.shape[1:]), z.dtype)
         for z in zero_outs])
    compiled = fast_dispatch_compile(
        lambda: jitted.lower(*example_in, *example_zeros).compile())
    return run


def kernel(**inputs):
    if "nc" not in _STATE:
        _STATE["nc"] = _build_nc()
    nc = _STATE["nc"]
    in_maps = _prep_inputs(inputs)
    if "runner" in _STATE:
        return _unshard(_STATE["runner"](in_maps))
    res = run_bass_kernel_spmd(nc, in_maps, list(range(NCORES))).results
    try:
        _STATE["runner"] = _make_runner(nc)
    except Exception:
        pass
    return _unshard([res[i]["out"] for i in range(NCORES)])


# revision 11
# speedup vs baseline: 1.0527x; 1.0527x over previous
"""Distributed Trainium2 kernel for nn_ArcTransformer (8 NeuronCores).

Algorithmic structure exploited (fixed problem shapes, V=16 vocab):
  * Every per-token q/k/v vector depends only on the token id (the MoE
    "compose" is position-independent), so the dense per-token expert MLP
    collapses to the 16 vocab rows.
  * Causal softmax attention over positions collapses to a cumulative
    token-count weighted sum over the 16 vocab classes:
        attn[t] = sum_v E[tok_t,v] * C[t,v] * v16[v] / sum_v E[tok_t,v]*C[t,v]
    with E = exp(scores between vocab rows), C = causal inclusive count
    of each vocab class up to position t.
  * Output projection + LM head fold into a single [16,16] matrix per head.

Sharding: data-parallel over batch, one full 2048-token row per core
(M=2). The kernel is tiny, so steady-state execute time is dominated by
per-core NEFF launch overhead, which the runtime serializes — measured
sustained per-execution time scales with core count (8 cores ~2.0 ms,
4 ~1.9 ms, 2 ~1.3 ms), so the widest shard that keeps batch-parallel
SPMD wins. Each core walks its row in four 512-position tiles, chaining
the class-count scan carry across tiles on device (carry-in at a row
start is zero). The only cross-head reduction is local (one K=128
matmul); no inter-core collective is needed — an on-device AllGather of
the outputs was measured SLOWER (per-iteration cross-core rendezvous).
Each core returns the logits for its row; the host concatenates.

Executable I/O is exactly ONE input operand and ONE output operand per
core (the unused partition-id input is disabled) — per-launch cost also
scales with the number of DRAM bindings, so everything the device needs,
tables AND constant masks, rides in one packed [128, 4546] fp16 input
(payload size is irrelevant next to binding count; the masks also drop
the gpsimd iota/affine_select ops, leaving 3 engine queues: sync DMA,
vector, tensor). Packed columns:
  *    0:16   rows 0:128 — folded value->logit table VO [128, 16];
  *   16:17   rows 0:128 — carry-in class counts (zero at row starts);
  *   17:145  rows 0:16  — per-head exp-score rows E (estk [16, 128]);
  *  145:161  rows 0:16  — folded residual logits embed @ head_w.T;
  *  161:2209 row  0     — the row's 2048 raw token ids as fp16;
  * 2210:4258 rows 0:16  — partition index (one-hot compare operand);
  * 4260:4388 rows 0:16  — tilew: eye(16) tiled across heads;
  * 4388:4516 rows 0:8   — bcw[h, p] = (p // 16 == h);
  * 4516:4524 rows 0:128 — denw = bcw.T;
  * 4524:4540 row  0     — ones[1, 16] (K=1 id-broadcast weights).
One-hot tokens: PE K=1 broadcast of the ids row, is_equal against the
partition-index block. Compute is all-fp32; DRAM I/O is fp16.

Device layout: [128, 512] tiles; partition p = h*16+v for head h and
vocab v; free dim = position within the current 512-wide tile.
"""

import sys

import numpy as np

sys.path.insert(0, "/opt/trn_rl_repo")

import jax  # noqa: E402

# The bass_exec HLO is deterministic, but each run_bass_kernel_spmd call
# jits a fresh closure, so the in-memory executable cache always misses
# and every call re-runs the ~350 ms neuronx compile hook. The persistent
# cache dedupes on HLO bytes and turns repeat calls into a disk hit.
for _opt, _val in (
    ("jax_compilation_cache_dir", "/tmp/jax_comp_cache"),
    ("jax_persistent_cache_min_compile_time_secs", 0.0),
    ("jax_persistent_cache_min_entry_size_bytes", 0),
):
    try:
        jax.config.update(_opt, _val)
    except Exception:
        pass

from concourse import bacc, mybir, tile  # noqa: E402
from concourse.bass_utils import run_bass_kernel_spmd  # noqa: E402

B, T, V, D = 2, 2048, 16, 512
NH, DH, P = 8, 64, 16
BT = B * T           # 4096 tokens
NCORES = 2           # data-parallel over batch: one row per core
CW = BT // NCORES    # 2048 tokens per core
TILE = 512           # free-dim tile width
NTILE = CW // TILE
F32 = mybir.dt.float32
F16 = mybir.dt.float16

# packed [128, INW] input column offsets (see module docstring)
IN_VO, IN_BASE, IN_ESTK, IN_XLT, IN_OH = 0, 16, 17, 145, 161
IN_TILEW = IN_OH + CW + 1       # 2210
IN_BCW = IN_TILEW + 128         # 2338
IN_DENW = IN_BCW + 128          # 2466
INW = IN_DENW + NH + 6          # 2480

_STATE = {}


def _build_nc():
    # target_bir_lowering=True lowers through AwsNeuronCustomNativeKernel:
    # stock neuronx-cc inlines the BIR into a regular NEFF instead of the
    # bass_exec custom-call NEFF wrapper. Measured: the wrapper pays a
    # ~0.5 ms/exec per-launch runtime tax this path does not (0.48 ms vs
    # 0.92 ms sustained per execution at 2 cores, identical numerics).
    nc = bacc.Bacc("TRN2", target_bir_lowering=True, debug=False,
                   num_devices=NCORES, enable_partition_id=False)

    inp_d = nc.declare_dram_parameter("inp", [128, INW], F16, isOutput=False)
    out_ext = nc.declare_dram_parameter("out", [V, CW], F16, isOutput=True)

    eq = mybir.AluOpType.is_equal
    add = mybir.AluOpType.add

    with tile.TileContext(nc) as tc:
        with (
            tc.tile_pool(name="sb", bufs=1) as sb,
            tc.tile_pool(name="ps", bufs=1, space="PSUM") as ps,
        ):
            # fp16 end to end on the PE datapath: one-hots and counts are
            # exact in fp16 (integers <= 2048), tables are fp16 payload
            # already, and PE fp16 matmuls avoid the 4-pass fp32r mode.
            inp_h = sb.tile([128, INW], F16)
            nc.sync.dma_start(inp_h[:], inp_d[:])

            vo = inp_h[0:128, IN_VO:IN_VO + V]        # value->logit [128,16]
            base128 = inp_h[0:128, IN_BASE:IN_BASE + 1]  # carry-in counts
            estk = inp_h[0:16, IN_ESTK:IN_ESTK + 128]    # E_h[u,v] at h*16+v
            xlt = inp_h[0:16, IN_XLT:IN_XLT + V]         # embed @ head_w.T
            oh_all = inp_h[0:16, IN_OH:IN_OH + CW]       # one-hot tokens
            tilew = inp_h[0:16, IN_TILEW:IN_TILEW + 128]
            bcw = inp_h[0:NH, IN_BCW:IN_BCW + 128]
            denw = inp_h[0:128, IN_DENW:IN_DENW + NH]

            cnt_all = sb.tile([128, CW], F16)
            g_all = sb.tile([128, CW], F16)
            gn_all = sb.tile([128, CW], F16)
            out_all = sb.tile([V, CW], F16)
            zero128 = sb.tile([128, TILE], F16)
            nc.vector.memset(zero128[:], 0.0)

            # walk the row in 512-wide tiles, chaining the count-scan carry
            carry = base128
            for j in range(NTILE):
                s = slice(j * TILE, (j + 1) * TILE)

                # one-hot tokens arrive as the input encoding of ids;
                # broadcast to all 8 head blocks (PE), then run
                # the inclusive count scan at 128 partitions straight out
                # of PSUM; fp16 counts are exact (integers <= 2048)
                oh = oh_all[0:16, s]
                ohb_ps = ps.tile([128, TILE], F32)
                nc.tensor.matmul(ohb_ps[:], tilew, oh)
                cnt = cnt_all[0:128, s]
                with nc.allow_low_precision(
                        reason="counts <= 2048 are exact in fp16"):
                    nc.vector.tensor_tensor_scan(
                        out=cnt, data0=ohb_ps[:], data1=zero128[:],
                        initial=carry, op0=add, op1=add)
                carry = cnt_all[0:128, (j + 1) * TILE - 1:(j + 1) * TILE]

                # G[h*16+v, t] = E_h[tok_t, v] * C[t, v]
                erow_ps = ps.tile([128, TILE], F32)
                nc.tensor.matmul(erow_ps[:], estk, oh)
                log_ps = ps.tile([V, TILE], F32)
                nc.tensor.matmul(log_ps[:], xlt, oh, start=True,
                                 stop=False)
                g = g_all[0:128, s]
                nc.vector.tensor_mul(g, erow_ps[:], cnt)

                # softmax denominator per head, reciprocal, broadcast back
                # (den <= t+1 <= 2048, so 1/den stays in fp16's normal
                # range and the fp16 reciprocal is safe)
                den_ps = ps.tile([NH, TILE], F32)
                nc.tensor.matmul(den_ps[:], denw, g)
                rec = sb.tile([NH, TILE], F16)
                with nc.allow_low_precision(
                        reason="1/den in [4.9e-4, 1], fp16 rel err 5e-4"):
                    nc.vector.reciprocal(rec[:], den_ps[:])
                bc_ps = ps.tile([128, TILE], F32)
                nc.tensor.matmul(bc_ps[:], bcw, rec[:])
                gn = gn_all[0:128, s]
                nc.vector.tensor_mul(gn, g, bc_ps[:])

                # logits[e,t] = sum_{h,v} VO[hv,e] * Gn[hv,t] + XL[tok_t,e]
                # (the XL term was accumulated into log_ps up front);
                # downcast on the vector engine — DMA cannot read PSUM, and
                # vector keeps the NEFF at 3 engine queues
                nc.tensor.matmul(log_ps[:], vo, gn, start=False, stop=True)
                nc.vector.tensor_tensor(out=out_all[0:V, s], in0=log_ps[:],
                                        in1=zero128[0:V, :], op=add)

            nc.sync.dma_start(out_ext[:], out_all[:])

    nc.compile()
    return nc


def _prep_inputs(inputs):
    ids = np.asarray(inputs["input_ids"]).astype(np.int64).reshape(BT)
    embed = np.asarray(inputs["embed"], dtype=np.float32)
    ln_g = np.asarray(inputs["ln_g"], dtype=np.float32)
    ln_b = np.asarray(inputs["ln_b"], dtype=np.float32)
    w1 = np.asarray(inputs["w1"], dtype=np.float32)
    w2 = np.asarray(inputs["w2"], dtype=np.float32)
    o_w = np.asarray(inputs["o_w"], dtype=np.float32)
    head_w = np.asarray(inputs["head_w"], dtype=np.float32)

    # LayerNorm of the 16 vocab embedding rows
    mu = embed.mean(axis=-1, keepdims=True)
    var = ((embed - mu) ** 2).mean(axis=-1, keepdims=True)
    h16 = (embed - mu) / np.sqrt(var + 1e-5) * ln_g + ln_b
    A = h16.reshape(V * NH, DH)                 # [128, 64] per-head rows

    scale = 1.0 / np.sqrt(DH)

    # expert MLP of the 16 vocab rows — shared by q/k/v (gate-independent)
    hmid = A @ w1.reshape(P * DH, DH).T         # [128, P*64]
    s = hmid * (1.0 / (1.0 + np.exp(-hmid)))    # silu
    s_p = np.ascontiguousarray(
        s.reshape(V * NH, P, DH).transpose(1, 0, 2))   # [P, 128, 64]
    outm = s_p @ w2.transpose(0, 2, 1)          # [P, 128, 64]

    def compose16(proto, gate):
        logits = (A @ np.asarray(proto, np.float32).T) * scale \
            - np.asarray(gate, np.float32)      # [128, P]
        w = np.where(logits > 1e-6, logits, 0.0).astype(np.float32)
        out = np.einsum("pxe,xp->xe", outm, w)  # [128, 64]
        return out.reshape(V, NH, DH).astype(np.float32)

    q16 = compose16(inputs["proto_q"], inputs["gate_q"])
    k16 = compose16(inputs["proto_k"], inputs["gate_k"])
    v16 = compose16(inputs["proto_v"], inputs["gate_v"])

    # per-head exp-score tables and folded value->logits matrices
    E_list, VO_list = [], []
    for h in range(NH):
        S = (q16[:, h, :] @ k16[:, h, :].T) * scale        # [16, 16]
        E_list.append(
            np.exp(S - S.max(axis=1, keepdims=True)).astype(np.float32))
        OW = o_w.T[h * DH:(h + 1) * DH, :] @ head_w.T       # [64, 16]
        VO_list.append((v16[:, h, :] @ OW).astype(np.float32))

    XL = embed @ head_w.T                       # [16, 16] residual-path logits

    estk = np.concatenate(E_list, axis=1)       # [16, 128]: E_h[u,v] @ h*16+v
    vo_st = np.concatenate(VO_list, axis=0)     # [128, 16]
    # one-hot encoding of the token ids (exact in fp16)
    oh_all = (ids[None, :] == np.arange(V)[:, None]).astype(np.float16)
    oh_all = oh_all.reshape(V, NCORES, CW)

    # constant masks, shipped once inside the packed input
    tilew = np.tile(np.eye(V, dtype=np.float16), (1, NH))    # [16, 128]
    bcw = np.zeros((NH, 128), np.float16)
    for h in range(NH):
        bcw[h, h * V:(h + 1) * V] = 1.0
    denw = bcw.T.copy()                                      # [128, 8]

    in_maps = []
    for i in range(NCORES):
        # each chunk starts at a batch-row boundary -> zero carry-in
        # (kept as an input column so the kernel stays general)
        inp = np.zeros((128, INW), np.float16)
        inp[:, IN_VO:IN_VO + V] = vo_st.astype(np.float16)
        inp[:, IN_BASE] = 0.0
        inp[0:V, IN_ESTK:IN_ESTK + 128] = estk.astype(np.float16)
        inp[0:V, IN_XLT:IN_XLT + V] = XL.astype(np.float16)
        inp[0:V, IN_OH:IN_OH + CW] = oh_all[:, i]
        inp[0:V, IN_TILEW:IN_TILEW + 128] = tilew
        inp[0:NH, IN_BCW:IN_BCW + 128] = bcw
        inp[:, IN_DENW:IN_DENW + NH] = denw
        in_maps.append({"inp": inp})
    return in_maps


def _unshard(per_core_out):
    # core i holds logits (vocab-major) for tokens [i*CW, (i+1)*CW)
    full = np.concatenate(
        [np.asarray(o, dtype=np.float32) for o in per_core_out], axis=1)
    return np.ascontiguousarray(full.T.reshape(B, T, V)).astype(np.float32)


def _make_runner(nc):
    """Persistent fast-dispatch executable for repeat kernel() calls —
    same bass_exec primitive run_bass_kernel_spmd lowers to, but compiled
    once and reused, so a second call skips the fresh-jit path."""
    import jax
    from jax.sharding import Mesh, PartitionSpec
    from jax.experimental.shard_map import shard_map
    from concourse import mybir as _mybir
    from concourse.bass2jax import (
        _bass_exec_p, fast_dispatch_compile, install_neuronx_cc_hook)

    install_neuronx_cc_hook()
    in_names, out_names, out_avals, zero_outs = [], [], [], []
    for alloc in nc.m.functions[0].allocations:
        if not isinstance(alloc, _mybir.MemoryLocationSet):
            continue
        name = alloc.memorylocations[0].name
        if alloc.kind == "ExternalInput":
            in_names.append(name)
        elif alloc.kind == "ExternalOutput":
            shape = tuple(alloc.tensor_shape)
            dtype = _mybir.dt.np(alloc.dtype)
            out_names.append(name)
            out_avals.append(jax.core.ShapedArray(shape, dtype))
            zero_outs.append(np.zeros(shape, dtype))
    n_params = len(in_names)
    all_names = in_names + out_names
    donate = tuple(range(n_params, n_params + len(out_avals)))

    def _body(*args):
        return tuple(_bass_exec_p.bind(
            *args, out_avals=tuple(out_avals), in_names=tuple(all_names),
            out_names=tuple(out_names), lowering_input_output_aliases=(),
            sim_require_finite=True, sim_require_nnan=True, nc=nc))

    from jax.sharding import NamedSharding
    mesh = Mesh(np.asarray(jax.devices()[:NCORES]), ("core",))
    shard = NamedSharding(mesh, PartitionSpec("core"))
    n_args = n_params + len(out_avals)
    jitted = jax.jit(
        shard_map(_body, mesh=mesh,
                  in_specs=(PartitionSpec("core"),) * n_args,
                  out_specs=(PartitionSpec("core"),) * len(out_names),
                  check_rep=False),
        donate_argnums=donate, keep_unused=True)

    def _place(arrays):
        return [jax.device_put(a, shard) for a in arrays]

    def run(in_maps):
        concat_in = [np.concatenate([np.asarray(m[name]) for m in in_maps], 0)
                     for name in in_names]
        zeros = [np.zeros((NCORES * z.shape[0], *z.shape[1:]), z.dtype)
                 for z in zero_outs]
        outs = compiled(*_place(concat_in), *_place(zeros))
        out = np.asarray(outs[0])
        return [out[i * V:(i + 1) * V] for i in range(NCORES)]

    example_in = _place([np.zeros((NCORES * 128, INW), np.float16)])
    example_zeros = _place(
        [np.zeros((NCORES * z.shape[0], *z.shape[1:]), z.dtype)
         for z in zero_outs])
    compiled = fast_dispatch_compile(
        lambda: jitted.lower(*example_in, *example_zeros).compile())
    return run


def kernel(**inputs):
    if "nc" not in _STATE:
        _STATE["nc"] = _build_nc()
    nc = _STATE["nc"]
    in_maps = _prep_inputs(inputs)
    if "runner" in _STATE:
        return _unshard(_STATE["runner"](in_maps))
    res = run_bass_kernel_spmd(nc, in_maps, list(range(NCORES))).results
    try:
        _STATE["runner"] = _make_runner(nc)
    except Exception:
        pass
    return _unshard([res[i]["out"] for i in range(NCORES)])
